# revision 1
# baseline (speedup 1.0000x reference)
"""Multi-head attention (S=4096, D=1024, H=16) on 8 trn2 NeuronCores.

Sharding: 2 heads per core (tensor-parallel on Q/K/V column splits and
dense row split). Each core computes a partial [S, D] output; host sums
the 8 partials (the unshard step for row-parallel TP).

Per-core layout (all fp32; matmuls run as float32r, N>=256):
  inputs : xT  [1024, 4096]  x transposed (same array on every core)
           wqT/wkT/wvT [1024, 128]  W[shard_rows].T for this core's 2 heads
           woT [128, 1024]          Wo[:, shard_cols].T
           ident [128, 128]         identity for PE transposes
  output : y [4096, 1024]           partial output

Per-core dataflow (heads h in {0,1}, dk=64):
  qT/kT [128(2h x 64d), 4096s]; v via vT chunks + PE transpose into
  v_aug [t-block, h, 65] (col 64 = ones -> softmax denominator rides the
  PV matmul). 8 s-waves of 512:
    LT[t, s0:s0+512] = k q^T per 128-t-block (K=64 row groups, both heads
    in one [128,1024] psum tile), PT = exp(0.125*LT) on ACT (no max
    subtraction: logits ~ N(0,1)), attnT_aug[65,512] += v_aug.T @ PT.
  Wave finalize: den -> reciprocal -> K=1 broadcast matmul -> normalize
  fused into the attnT copy -> y = attnT.T @ woT -> DMA out. PV and
  finalize run through a lagged FIFO so the in-order PE queue never
  stalls on unreleased PSUM slots or DVE-latency chains; the first 12
  attention iterations interleave with the projection phase.
"""

import numpy as np
from collections import deque
from contextlib import ExitStack

S = 4096
D = 1024
NCORES = 8
HD = 128  # head-dim span per core (2 heads x 64)
DK = 64

_NC_CACHE = {}


def _split_multi_waits(nc, mybir):
    """This walrus build encodes at most ~2 sync commands per instruction
    (1 for matmul/drain). Keep <=1 wait on every compute/DMA instruction and
    move the rest into standalone dual-condition EventSemaphore instructions
    inserted immediately before it on the same engine (same wait point, so
    semantics are unchanged)."""
    n = 0
    used = set()
    for b in nc.m.functions[0].blocks:
        for inst in b.instructions:
            si = inst.sync_info
            if si:
                for w in (si.on_wait or []):
                    used.add(w.id)
                for u in (si.on_update or []):
                    used.add(u.id)
    free_ids = [i for i in range(max(used) + 1, max(used) + 32)]
    sems = {}

    def eng_sem(eng):
        if eng not in sems:
            sems[eng] = (free_ids.pop(0), f"wsplit_{len(sems)}")
        return sems[eng]

    for b in nc.m.functions[0].blocks:
        il = b.instructions
        new = []
        for inst in il:
            si = inst.sync_info
            waits = list(si.on_wait) if si and si.on_wait else []
            upds = list(si.on_update) if si and si.on_update else []
            if type(inst).__name__ == "InstEventSemaphore":
                new.append(inst)
                continue
            if len(waits) > 1:
                excess, keep = waits[:-1], waits[-1:]
                for i in range(0, len(excess), 2):
                    sid, sname = eng_sem(inst.engine)
                    ev = mybir.InstEventSemaphore(
                        name=f"{inst.name}_ws{i}", engine=inst.engine,
                        ins=[], outs=[],
                        sync_info=mybir.SyncInfo(
                            on_wait=excess[i:i + 2],
                            on_update=[mybir.SyncUpdate(
                                sync_type="semaphore", id=sid,
                                ant_name=sname, update_mode="sem-inc",
                                update_value=1, update_reg=None)]))
                    new.append(ev)
                    n += 1
                inst.sync_info = mybir.SyncInfo(on_wait=keep, on_update=upds)
            new.append(inst)
        il[:] = new
    return n


def _build_nc():
    import concourse.bass as bass
    import concourse.tile as tile
    import concourse.mybir as mybir

    fp32 = mybir.dt.float32
    f32r = mybir.dt.float32r
    AF = mybir.ActivationFunctionType

    nc = bass.Bass()
    xT = nc.dram_tensor("xT", [D, S], f32r, kind="ExternalInput")
    wqT = nc.dram_tensor("wqT", [D, HD], f32r, kind="ExternalInput")
    wkT = nc.dram_tensor("wkT", [D, HD], f32r, kind="ExternalInput")
    wvT = nc.dram_tensor("wvT", [D, HD], f32r, kind="ExternalInput")
    woT = nc.dram_tensor("woT", [HD, D], f32r, kind="ExternalInput")
    ident = nc.dram_tensor("ident", [128, 128], f32r, kind="ExternalInput")
    ones_in = nc.dram_tensor("ones_in", [128, DK], f32r, kind="ExternalInput")
    y = nc.dram_tensor("y", [S, D], fp32, kind="ExternalOutput")

    NT = S // 128   # 32 t-blocks
    NWV = 8         # s-waves of 512

    with tile.TileContext(nc) as tc, ExitStack() as ctx, \
         nc.allow_low_precision(reason="float32r matmul operand rounding"):
        sb = ctx.enter_context(tc.tile_pool(name="sb", bufs=1))

        qT_sb = sb.tile([128, S], f32r, tag="qT")
        kT_sb = sb.tile([128, S], f32r, tag="kT")
        # v_aug[:, tb, h, 0:64] = v block for head h; [..., 64] = ones
        v_aug = sb.tile([128, NT, 2, DK + 1], f32r, tag="vaug")
        attnT = sb.tile([128, S], f32r, tag="attnT")
        ones_r = sb.tile([1, DK], f32r, tag="onesr")
        warm = sb.tile([1, DK], fp32, tag="warm")
        wo_sb = sb.tile([HD, D], f32r, tag="wo")
        id_sb = sb.tile([128, 128], f32r, tag="id")

        nc.sync.dma_start(wo_sb[:], woT[:])
        nc.sync.dma_start(id_sb[:], ident[:])
        # f32r constants must come from DRAM (memset cannot round to f32r)
        nc.sync.dma_start(ones_r[:], ones_in[0:1, :])
        nc.sync.dma_start(
            v_aug[:, :, :, DK],
            ones_in[:, 0:DK].rearrange("p (a b) -> p a b", a=NT))
        # preload the exp table set on ACT before the first real exp
        nc.scalar.activation(warm[:], ones_r[:], AF.Exp, scale=0.125)

        proj_ctx = ExitStack()
        with tc.tile_pool(name="wpool", bufs=1) as wpool, \
             tc.tile_pool(name="xpool", bufs=2) as xpool, \
             tc.tile_pool(name="ltpool", bufs=2, space="PSUM") as ltpool, \
             tc.tile_pool(name="ptpool", bufs=12) as ptpool, \
             tc.tile_pool(name="opool", bufs=3) as opool, \
             tc.tile_pool(name="dpool", bufs=2) as dpool:
            ppsum = proj_ctx.enter_context(
                tc.tile_pool(name="ppsum", bufs=3, space="PSUM"))
            tpsum = proj_ctx.enter_context(
                tc.tile_pool(name="tpsum", bufs=1, space="PSUM"))

            wq_sb = wpool.tile([128, 8, HD], f32r, tag="wq")
            wk_sb = wpool.tile([128, 8, HD], f32r, tag="wk")
            wv_sb = wpool.tile([128, 8, HD], f32r, tag="wv")
            nc.sync.dma_start(wq_sb[:], wqT.rearrange("(c p) m -> p c m", p=128))
            nc.sync.dma_start(wk_sb[:], wkT.rearrange("(c p) m -> p c m", p=128))
            nc.sync.dma_start(wv_sb[:], wvT.rearrange("(c p) m -> p c m", p=128))
            xTr = xT.rearrange("(c p) s -> p c s", p=128)

            # ---------- projections for one half-quarter (s/t range p*512) ----
            def proj_piece(p):
                xq = xpool.tile([128, 8, 512], f32r, tag="xq", name=f"xq_{p}")
                for c in range(0, 8, 2):
                    nc.sync.dma_start(xq[:, c:c + 2, :],
                                      xTr[:, c:c + 2, p * 512:(p + 1) * 512])
                s0 = p * 512
                pk = ppsum.tile([128, 512], fp32, tag="proj", name=f"pk_{p}")
                for c in range(8):
                    nc.tensor.matmul(pk[:], wk_sb[:, c, :], xq[:, c, :],
                                     start=(c == 0), stop=(c == 7))
                nc.any.tensor_copy(kT_sb[:, s0:s0 + 512], pk[:])
                pq = ppsum.tile([128, 512], fp32, tag="proj", name=f"pq_{p}")
                for c in range(8):
                    nc.tensor.matmul(pq[:], wq_sb[:, c, :], xq[:, c, :],
                                     start=(c == 0), stop=(c == 7))
                nc.any.tensor_copy(qT_sb[:, s0:s0 + 512], pq[:])
                pvT = ppsum.tile([128, 512], fp32, tag="proj", name=f"pv_{p}")
                for c in range(8):
                    nc.tensor.matmul(pvT[:], wv_sb[:, c, :], xq[:, c, :],
                                     start=(c == 0), stop=(c == 7))
                vtmp = xpool.tile([128, 512], f32r, tag="vtmp", name=f"vtmp_{p}")
                nc.any.tensor_copy(vtmp[:], pvT[:])
                for i in range(4):
                    tb = p * 4 + i
                    pvt = tpsum.tile([128, 128], f32r, tag="projvt",
                                     name=f"pvt_{tb}")
                    nc.tensor.transpose(pvt[:],
                                        vtmp[:, i * 128:(i + 1) * 128],
                                        id_sb[:])
                    nc.any.tensor_copy(
                        v_aug[:, tb, :, 0:DK],
                        pvt[:].rearrange("p (h d) -> p h d", h=2))

            # ---------- attention machinery ----------
            LAG = 4
            pending = deque()  # (ready_gi, is_pv, thunk)
            state = {"gi": 0, "maxpv": 0}
            lt_holder = {}

            def emit_lt(w, tb):
                s0 = w * 512
                lt = ltpool.tile([128, 1024], fp32, tag="lt",
                                 name=f"lt_{w}_{tb}")
                for h in range(2):
                    nc.tensor.matmul(
                        lt[:, h * 512:(h + 1) * 512],
                        kT_sb[DK * h:DK * (h + 1), tb * 128:(tb + 1) * 128],
                        qT_sb[DK * h:DK * (h + 1), s0:s0 + 512],
                        start=True, stop=True,
                        tile_position=(DK * h, 0),
                    )
                return lt

            def pv_thunk(w, tb, pt, accs):
                def run():
                    if tb == 0:
                        accs.extend(
                            accpool.tile([DK + 1, 512], fp32, tag="acc",
                                         name=f"acc_{w}_{h}") for h in range(2))
                    for h in range(2):
                        nc.tensor.matmul(
                            accs[h][:],
                            v_aug[:, tb, h, :],
                            pt[:, h * 512:(h + 1) * 512],
                            start=(tb == 0), stop=(tb == NT - 1),
                        )
                return run

            def finalize_thunks(w, accs):
                s0 = w * 512
                denw = dpool.tile([1, 1024], fp32, tag="denw", name=f"den_{w}")
                rdenw = dpool.tile([1, 1024], f32r, tag="rdenw", name=f"rden_{w}")

                def den_recip():
                    for h in range(2):
                        nc.vector.tensor_copy(denw[0:1, h * 512:(h + 1) * 512],
                                              accs[h][DK:DK + 1, :])
                    nc.vector.reciprocal(rdenw[:], denw[:])

                def norm(h):
                    bc = ypool.tile([DK, 512], fp32, tag="y", name=f"bc_{w}_{h}")
                    nc.tensor.matmul(
                        bc[:], ones_r[:],
                        rdenw[0:1, h * 512:(h + 1) * 512],
                        start=True, stop=True)
                    # walrus rejects tensor_tensor with two PSUM operands
                    bcs = opool.tile([DK, 512], fp32, tag="bcs",
                                     name=f"bcs_{w}_{h}")
                    nc.vector.tensor_copy(bcs[:], bc[:])
                    nc.vector.tensor_mul(attnT[DK * h:DK * (h + 1), s0:s0 + 512],
                                         accs[h][0:DK, :], bcs[:])

                def yblock(bl, jc):
                    b = w * 4 + bl
                    yp = ypool.tile([128, 512], fp32, tag="y",
                                    name=f"yp_{b}_{jc}")
                    nc.tensor.matmul(
                        yp[:],
                        attnT[:, b * 128:(b + 1) * 128],
                        wo_sb[:, jc * 512:(jc + 1) * 512],
                        start=True, stop=True)
                    yo = opool.tile([128, 512], fp32, tag="yo",
                                    name=f"yo_{b}_{jc}")
                    nc.vector.tensor_copy(yo[:], yp[:])
                    nc.sync.dma_start(
                        y[b * 128:(b + 1) * 128, jc * 512:(jc + 1) * 512],
                        yo[:])

                thunks = [den_recip, lambda: norm(0), lambda: norm(1)]
                for bl in range(4):
                    for jc in range(2):
                        thunks.append(lambda bl=bl, jc=jc: yblock(bl, jc))
                return thunks

            def emit_iter(w, tb, accs, ready_floor=0):
                gi = state["gi"]
                lt = lt_holder.pop("lt")
                pt = ptpool.tile([128, 1024], f32r, tag="pt",
                                 name=f"pt_{w}_{tb}")
                nc.scalar.activation(pt[:], lt[:], AF.Exp, scale=0.125)
                if tb + 1 < NT:
                    lt_holder["lt"] = emit_lt(w, tb + 1)
                elif w + 1 < NWV:
                    lt_holder["lt"] = emit_lt(w + 1, 0)
                pending.append((max(gi + LAG, ready_floor), True,
                                pv_thunk(w, tb, pt, accs)))
                state["maxpv"] = max(state["maxpv"],
                                     sum(1 for e in pending if e[1]))
                pops = 0
                while pending and pending[0][0] <= gi and pops < 2:
                    pending.popleft()[2]()
                    pops += 1
                state["gi"] = gi + 1

            # ---------- emission: proj with first attention iters woven in ----
            proj_piece(0)
            proj_piece(1)
            lt_holder["lt"] = emit_lt(0, 0)
            acc0 = []
            tb0 = 0
            for p in range(2, 8):
                proj_piece(p)
                emit_iter(0, tb0, acc0, ready_floor=8)
                tb0 += 1
            proj_ctx.close()
            att_ctx = ExitStack()
            accpool = att_ctx.enter_context(
                tc.tile_pool(name="accpool", bufs=3, space="PSUM"))
            ypool = att_ctx.enter_context(
                tc.tile_pool(name="ypool", bufs=1, space="PSUM"))
            for tb in range(tb0, NT):
                emit_iter(0, tb, acc0)
            for t in finalize_thunks(0, acc0):
                pending.append((state["gi"] + LAG - 1, False, t))

            for w in range(1, NWV):
                accs = []
                for tb in range(NT):
                    emit_iter(w, tb, accs)
                for t in finalize_thunks(w, accs):
                    pending.append((state["gi"] + LAG - 1, False, t))
            while pending:
                pending.popleft()[2]()
            assert state["maxpv"] <= 10, f"pt pool too small: {state['maxpv']}"
            att_ctx.close()

    _split_multi_waits(nc, mybir)
    nc.finalize()
    return nc


def _get_nc():
    if "nc" not in _NC_CACHE:
        _NC_CACHE["nc"] = _build_nc()
    return _NC_CACHE["nc"]


def _in_maps(x, Wq, Wk, Wv, Wo):
    xT = np.ascontiguousarray(x.T).astype(np.float32, copy=False)
    ident = np.eye(128, dtype=np.float32)
    maps = []
    for c in range(NCORES):
        sl = slice(HD * c, HD * (c + 1))
        maps.append(dict(
            xT=xT,
            wqT=np.ascontiguousarray(Wq[sl, :].T),
            wkT=np.ascontiguousarray(Wk[sl, :].T),
            wvT=np.ascontiguousarray(Wv[sl, :].T),
            woT=np.ascontiguousarray(Wo[:, sl].T),
            ident=ident,
            ones_in=np.ones((128, DK), dtype=np.float32),
        ))
    return maps


def kernel(x, Wq, Wk, Wv, Wo):
    from concourse.bass_utils import run_bass_kernel_spmd

    x = np.asarray(x, dtype=np.float32)
    nc = _get_nc()
    res = run_bass_kernel_spmd(nc, _in_maps(x, Wq, Wk, Wv, Wo),
                               list(range(NCORES)))
    out = np.zeros((S, D), np.float32)
    for rr in res.results:
        out += rr["y"]
    return out

